# revision 8
# baseline (speedup 1.0000x reference)
"""Multi-head cross-attention Trainium2 Bass kernel, SPMD over 8 NeuronCores.

Sharding: core c handles batch b = c//2 and head group g = c%2 (8 of 16 heads).
Each core computes a partial output projection (its heads' W_o rows); the host
sums the two partials per batch element.

Device pipeline per core (all matmuls bf16 with fp32 PSUM accumulation):
  kT = (Wk^T x^T)          [512 hd, 2048 kseq]   (per-partition bias b_k)
  v  = (x Wv)              [2048 kseq, 8*65]     (65th col per head = ones)
  qT = (Wq^T y^T)          [512 hd, 1024 q]      (per-partition bias b_q)
  per (head-pair, q-tile, k-chunk):
      S^T[k, q|q'] = kT_h^T-chunk @ qT_h for both heads of the pair
        (K=64 row-tiled at partitions 0/64 -> the two matmuls run
         concurrently in the PE array; both write one 2-bank PSUM tile)
      em = exp(0.125 * S^T)  (one ACT op per pair; no row-max: |S|<=~25)
      em *= maskT            (one DVE mul per pair; multiplicative masking
                              == additive -inf pre-exp)
      acc_h[65, q] += [v_h | 1]^T @ em_h   (row 64 = softmax denominator)
  normalize: vals_h = acc[0:64] * bcast(1/acc[64])  (PE outer-product bcast)
  out_partial = vals^T-chunks @ Wo-rows  -> [1024 q, 1024 D] fp32

Engine assignment keeps ACT exp-only (the per-iteration gate is the
[128,1024] exp at ~1.1us): PSUM->SBUF copies for v and the output
projection, and the deferred projection bias-adds, run on the idle Pool
(gpsimd) engine; DVE carries the mask multiplies and normalize chain.
All projection work that is not needed for the first attention iterations
is deferred into the attention loop with per-op deadlines (global iter
index by which the op must have issued) so the PE never starves while the
input DMA streams in.  b_v and b_o fold into a host-side constant row.
"""

import sys

import numpy as np
import ml_dtypes

if "/opt/trn_rl_repo" not in sys.path:
    sys.path.insert(0, "/opt/trn_rl_repo")

BF = ml_dtypes.bfloat16

B, NKV, NQ, D, H = 4, 2048, 1024, 1024, 16
HD = D // H          # 64
NHL = 8              # heads per core (local)
P = 128
DC = D // P          # 8 contraction chunks over model dim
KC = NKV // P        # 16 key-seq chunks
QT = NQ // 512       # 2 q tiles of 512 for attention
MT = 4               # hd-dim chunks of kT/qT (512/128)

_CACHE = {}


def _build_program():
    import concourse.bass as bass
    import concourse.mybir as mybir
    import concourse.tile as tile
    from concourse import bacc

    f32 = mybir.dt.float32
    bf16 = mybir.dt.bfloat16

    nc = bacc.Bacc(
        "TRN2", target_bir_lowering=False, debug=False, num_devices=8
    )

    xT_d = nc.dram_tensor("xT", [D, NKV], bf16, kind="ExternalInput").ap()
    yT_d = nc.dram_tensor("yT", [D, NQ], bf16, kind="ExternalInput").ap()
    maskT_d = nc.dram_tensor("maskT", [NKV, NQ], bf16, kind="ExternalInput").ap()
    wk_d = nc.dram_tensor("wk", [D, 512], bf16, kind="ExternalInput").ap()
    wv_d = nc.dram_tensor("wv", [D, 512], bf16, kind="ExternalInput").ap()
    wq_d = nc.dram_tensor("wq", [D, 512], bf16, kind="ExternalInput").ap()
    wo_d = nc.dram_tensor("wo", [512, D], bf16, kind="ExternalInput").ap()
    bk_d = nc.dram_tensor("bk", [512, 1], f32, kind="ExternalInput").ap()
    bq_d = nc.dram_tensor("bq", [512, 1], f32, kind="ExternalInput").ap()
    out_d = nc.dram_tensor("out", [NQ, D], bf16, kind="ExternalOutput").ap()

    Exp = mybir.ActivationFunctionType.Exp

    with tile.TileContext(nc) as tc:
        with (
            tc.tile_pool(name="persist", bufs=1) as persist,
            tc.tile_pool(name="work", bufs=3) as work,
            tc.tile_pool(name="empool", bufs=4) as empool,
            tc.tile_pool(name="pmm", bufs=2, space="PSUM") as pmm,
            tc.tile_pool(name="pacc", bufs=2, space="PSUM") as pacc,
            tc.tile_pool(name="psc", bufs=2, space="PSUM") as psc,
        ):
            def row_tile(nchunks, cols, dtype, label):
                return [
                    persist.tile(
                        [P, cols], dtype, tag=f"{label}{i}", name=f"{label}{i}"
                    )
                    for i in range(nchunks)
                ]

            def load(tiles, dram, i):
                nc.sync.dma_start(tiles[i], dram[i * P:(i + 1) * P, :])

            wk_sb = row_tile(DC, 512, bf16, "wk")
            wv_sb = row_tile(DC, 512, bf16, "wv")
            xT_sb = row_tile(DC, NKV, bf16, "xT")
            wq_sb = row_tile(DC, 512, bf16, "wq")
            yT_sb = row_tile(DC, NQ, bf16, "yT")
            bk_sb = row_tile(MT, 1, f32, "bk")
            bq_sb = row_tile(MT, 1, f32, "bq")
            maskT_sb = row_tile(KC, NQ, bf16, "mT")
            wo_sb = row_tile(MT, D, bf16, "wo")

            def load_xt_q(d, qb):
                nc.sync.dma_start(
                    xT_sb[d][:, qb * 512:(qb + 1) * 512],
                    xT_d[d * P:(d + 1) * P, qb * 512:(qb + 1) * 512],
                )

            def load_yt_h(d, h):
                nc.sync.dma_start(
                    yT_sb[d][:, h * 512:(h + 1) * 512],
                    yT_d[d * P:(d + 1) * P, h * 512:(h + 1) * 512],
                )

            # DMA issue order = consumption order.  The attention loop's
            # first iterations need kT m0 n0 (wk + xT q0), v0 (wv), qT m0 n0
            # (wq + yT half0) and mask kc0 — so those stream first; the rest
            # interleaves by first-use deadline so the loop never outruns it.
            for m in range(MT):
                load(bk_sb, bk_d, m)
                load(bq_sb, bq_d, m)
            for d in range(DC):
                load(wk_sb, wk_d, d)
                load_xt_q(d, 0)
            for d in range(DC):
                load(wv_sb, wv_d, d)
            for d in range(DC):
                load(wq_sb, wq_d, d)
                load_yt_h(d, 0)
            for i in range(0, 4):
                load(maskT_sb, maskT_d, i)
            for d in range(DC):
                load_xt_q(d, 1)
            for i in range(4, 8):
                load(maskT_sb, maskT_d, i)
            for d in range(DC):
                load_xt_q(d, 2)
            for i in range(8, 12):
                load(maskT_sb, maskT_d, i)
            for d in range(DC):
                load_xt_q(d, 3)
            for d in range(DC):
                load_yt_h(d, 1)
            for i in range(12, 16):
                load(maskT_sb, maskT_d, i)
            for m in range(MT):
                load(wo_sb, wo_d, m)

            ones_sb = persist.tile([1, HD], bf16, tag="ones", name="ones")
            nc.gpsimd.memset(ones_sb, 1.0)

            kT_sb = [
                persist.tile([P, NKV], bf16, tag=f"kT{m}", name=f"kT{m}")
                for m in range(MT)
            ]
            qT_sb = [
                persist.tile([P, NQ], bf16, tag=f"qT{m}", name=f"qT{m}")
                for m in range(MT)
            ]
            v_sb = [
                persist.tile([P, NHL * 65], bf16, tag=f"v{i}", name=f"v{i}")
                for i in range(KC)
            ]
            for i in range(KC):
                nc.gpsimd.memset(
                    v_sb[i].rearrange("p (h c) -> p h c", c=65)[:, :, 64:65], 1.0
                )
            vals_sb = [
                persist.tile([P, NQ], bf16, tag=f"vals{c}", name=f"vals{c}")
                for c in range(MT)
            ]

            # ---- projection op closures -------------------------------
            # One closure per matmul; the chain's last op appends the
            # bias-add (DVE while the loop hasn't started, Pool once the
            # attention loop is live so DVE/ACT stay on the critical path).
            def proj_ops(m, which, ns, bias_eng):
                w_sb, dst, bias, src = (
                    (wk_sb, kT_sb, bk_sb, xT_sb) if which == "k"
                    else (wq_sb, qT_sb, bq_sb, yT_sb)
                )
                ops = []
                hold = {}
                for n in ns:
                    for d in range(DC):
                        def op(m=m, n=n, d=d):
                            if d == 0:
                                hold[n] = pmm.tile(
                                    [P, 512], f32, tag="mm",
                                    name=f"pj{which}{m}_{n}"
                                )
                            nc.tensor.matmul(
                                hold[n],
                                lhsT=w_sb[d][:, m * P:(m + 1) * P],
                                rhs=src[d][:, n * 512:(n + 1) * 512],
                                start=(d == 0),
                                stop=(d == DC - 1),
                            )
                            if d == DC - 1:
                                bias_eng().tensor_scalar_add(
                                    dst[m][:, n * 512:(n + 1) * 512],
                                    hold[n], bias[m]
                                )
                        ops.append(op)
                return ops

            def v_ops(i):
                # v chunk i = x[i*128:(i+1)*128] @ Wv as 8 matmuls + a Pool
                # copy (PSUM fp32 -> strided bf16 cols, ones col untouched)
                hold = {}
                ops = []
                for d in range(DC):
                    def op(i=i, d=d):
                        if d == 0:
                            hold[0] = pmm.tile(
                                [P, 512], f32, tag="mm", name=f"ps_v{i}"
                            )
                        nc.tensor.matmul(
                            hold[0],
                            lhsT=xT_sb[d][:, i * P:(i + 1) * P],
                            rhs=wv_sb[d],
                            start=(d == 0),
                            stop=(d == DC - 1),
                        )
                        if d == DC - 1:
                            v3 = v_sb[i].rearrange("p (h c) -> p h c", c=65)
                            nc.vector.tensor_copy(
                                v3[:, :, 0:64],
                                hold[0].rearrange("p (h c) -> p h c", c=64),
                            )
                    ops.append(op)
                return ops

            # ---- upfront: only what the first attention iters need ----
            for op in proj_ops(0, "k", [0], lambda: nc.vector):
                op()
            for i in range(4):
                for op in v_ops(i):
                    op()
            for op in proj_ops(0, "q", [0], lambda: nc.vector):
                op()

            # ---- deferred work, drained inside the attention loop -----
            # (deadline, op): deadline = global iter (hp*32 + t*16 + kc) by
            # which the op must have issued.  In-queue order must be
            # non-decreasing in deadline.
            gp = lambda: nc.vector
            deferred = []

            def defer(deadline, ops):
                for op in ops:
                    deferred.append((deadline, op))

            # hp0/t0 just-in-time: kT m0 col-chunks + v chunks, in kc order
            defer(2, proj_ops(0, "k", [1], gp))        # kc 4..7
            defer(3, v_ops(4))
            defer(4, v_ops(5))
            defer(5, v_ops(6))
            defer(6, v_ops(7))
            defer(7, proj_ops(0, "k", [2], gp))        # kc 8..11
            defer(8, v_ops(8))
            defer(9, v_ops(9))
            defer(10, v_ops(10))
            defer(11, proj_ops(0, "k", [3], gp))       # kc 12..15
            defer(12, v_ops(11))
            defer(13, v_ops(12))
            defer(13, v_ops(13))
            defer(14, v_ops(14))
            defer(14, v_ops(15))
            defer(15, proj_ops(0, "q", [1], gp))       # hp0 t1
            # m1 (hp1) / m2 (hp2) / m3 (hp3): each n-chunk by first use
            defer(28, proj_ops(1, "k", [0], gp))
            defer(30, proj_ops(1, "q", [0], gp))
            defer(35, proj_ops(1, "k", [1], gp))
            defer(39, proj_ops(1, "k", [2], gp))
            defer(43, proj_ops(1, "k", [3], gp))
            defer(46, proj_ops(1, "q", [1], gp))
            defer(60, proj_ops(2, "k", [0], gp))
            defer(62, proj_ops(2, "q", [0], gp))
            defer(67, proj_ops(2, "k", [1], gp))
            defer(71, proj_ops(2, "k", [2], gp))
            defer(75, proj_ops(2, "k", [3], gp))
            defer(78, proj_ops(2, "q", [1], gp))
            defer(92, proj_ops(3, "k", [0], gp))
            defer(94, proj_ops(3, "q", [0], gp))
            defer(99, proj_ops(3, "k", [1], gp))
            defer(103, proj_ops(3, "k", [2], gp))
            defer(107, proj_ops(3, "k", [3], gp))
            defer(110, proj_ops(3, "q", [1], gp))

            dq = list(reversed(deferred))  # pop() from the end = FIFO

            def drain(g, slots_left):
                # everything at/near its deadline, then even spread
                n = 0
                while dq and dq[-1][0] <= g + 2:
                    dq.pop()[1]()
                    n += 1
                want = -(-len(dq) // max(1, slots_left))
                while n < want and dq:
                    dq.pop()[1]()
                    n += 1

            # ---- attention ------------------------------------------------
            norm_pending = []

            def make_norm(hp, t, a, h, ut):
                po = a * HD
                qs = slice(t * 512, (t + 1) * 512)

                def norm_op(bps_pool=pmm, bps_tag="mm"):
                    # partition-shifting copy (64 -> 0) is fine on DVE, but
                    # reciprocal_approx_fast is lane-aligned only
                    s_f = work.tile([1, 512], f32, tag="s", name=f"s{h}_{t}")
                    nc.vector.tensor_copy(s_f, ut[HD:HD + 1, :])
                    r_f = work.tile([1, 512], f32, tag="r", name=f"r{h}_{t}")
                    nc.vector.reciprocal_approx_fast(r_f, s_f)
                    r_b = work.tile([1, 512], bf16, tag="rb", name=f"rb{h}_{t}")
                    nc.vector.tensor_copy(r_b, r_f)
                    bps = bps_pool.tile(
                        [HD, 512], f32, tag=bps_tag, name=f"bps{h}_{t}"
                    )
                    nc.tensor.matmul(
                        bps, lhsT=ones_sb, rhs=r_b, start=True, stop=True
                    )
                    nc.vector.tensor_mul(
                        vals_sb[hp][po:po + HD, qs], ut[0:HD, :], bps
                    )
                return norm_op

            def wo_ops(t2s, pool_pick, copy_fn):
                # output-projection chains as per-matmul closures; the last
                # op of a chain appends the Pool PSUM->SBUF copy + out DMA
                ops = []
                hold = {}
                for t2 in t2s:
                    for n in range(D // 512):
                        for c in range(MT):
                            def op(t2=t2, n=n, c=c):
                                if c == 0:
                                    pool, tag = pool_pick(t2, n)
                                    hold[(t2, n)] = pool.tile(
                                        [P, 512], f32, tag=tag,
                                        name=f"ps_o{t2}_{n}"
                                    )
                                ps_o = hold[(t2, n)]
                                nc.tensor.matmul(
                                    ps_o,
                                    lhsT=vals_sb[c][:, t2 * P:(t2 + 1) * P],
                                    rhs=wo_sb[c][:, n * 512:(n + 1) * 512],
                                    start=(c == 0),
                                    stop=(c == MT - 1),
                                )
                                if c == MT - 1:
                                    ot = work.tile(
                                        [P, 512], bf16, tag="ot",
                                        name=f"ot{t2}_{n}", bufs=3
                                    )
                                    copy_fn(ot, ps_o)
                                    nc.sync.dma_start(
                                        out_d[t2 * P:(t2 + 1) * P,
                                              n * 512:(n + 1) * 512], ot
                                    )
                            ops.append(op)
                return ops

            # first q-half of the output projection drains inside the last
            # attention block (all t=0 norms final by then)
            wo_first = list(reversed(
                wo_ops(range(0, 4), lambda t2, n: (pmm, "mm"),
                       lambda o, i: nc.vector.tensor_copy(o, i))
            ))

            for hp in range(NHL // 2):
                h0, h1 = 2 * hp, 2 * hp + 1
                for t in range(QT):
                    qs = slice(t * 512, (t + 1) * 512)
                    accs = [
                        pacc.tile([65, 512], f32, tag="acc", name=f"acc{h}_{t}")
                        for h in (h0, h1)
                    ]
                    for kc in range(KC):
                        g = hp * 32 + t * 16 + kc
                        drain(g, 127 - g)
                        if kc % 4 == 2 and norm_pending:
                            norm_pending.pop(0)()
                        if hp == 3 and t == 1 and kc >= 8:
                            for _ in range(4):
                                if wo_first:
                                    wo_first.pop()()

                        sp2 = psc.tile(
                            [P, 1024], f32, tag="sc", name=f"sp{hp}_{t}_{kc}"
                        )
                        for a in range(2):
                            po = a * HD
                            nc.tensor.matmul(
                                sp2[:, a * 512:(a + 1) * 512],
                                lhsT=kT_sb[hp][po:po + HD, kc * P:(kc + 1) * P],
                                rhs=qT_sb[hp][po:po + HD, qs],
                                start=True,
                                stop=True,
                            )
                        em2 = empool.tile(
                            [P, 1024], bf16, tag="em", name=f"em{hp}_{t}_{kc}"
                        )
                        nc.scalar.activation(em2, sp2, Exp, scale=0.125)
                        # one masked multiply for both heads: the mask chunk
                        # is read once via a step-0 broadcast dim
                        mb = (maskT_sb[kc][:, qs]
                              .rearrange("p (o q) -> p o q", o=1)
                              .broadcast_to([P, 2, 512]))
                        em3 = em2.rearrange("p (o q) -> p o q", o=2)
                        nc.vector.tensor_mul(em3, em3, mb)
                        for a, h in enumerate((h0, h1)):
                            nc.tensor.matmul(
                                accs[a],
                                lhsT=v_sb[kc][:, h * 65:(h + 1) * 65],
                                rhs=em2[:, a * 512:(a + 1) * 512],
                                start=(kc == 0),
                                stop=(kc == KC - 1),
                            )
                    for a, h in enumerate((h0, h1)):
                        # single [65,512] DVE copy frees the PSUM accumulator
                        # (row 64 = softmax denominator rides along); the
                        # normalize is deferred into the next block.
                        ut = work.tile(
                            [HD + 1, 512], f32, tag="ut", name=f"ut{h}_{t}",
                            bufs=5
                        )
                        nc.vector.tensor_copy(ut, accs[a][0:HD + 1, :])
                        norm_pending.append(make_norm(hp, t, a, h, ut))

            while wo_first:
                wo_first.pop()()

            # ---- output projection, second q-half ----
            # Chains j=1,3 (psc ring) pre-emit c=0..2 to cover the final
            # norms' DVE chain; bps tiles take the free pmm ring (no wo
            # chain holds it pre-norm, so no ring cycle).
            ops2 = wo_ops(
                range(4, NQ // P),
                lambda t2, n: ((psc, "sc") if (t2 * 2 + n) % 2 == 1
                               else (pmm, "mm")),
                lambda o, i: nc.scalar.copy(o, i),
            )
            for j in (1, 3):
                for c in range(3):
                    ops2[j * MT + c]()
            while norm_pending:
                norm_pending.pop(0)()
            for j in (1, 3):
                ops2[j * MT + 3]()
            for j in (0, 2, 5, 7, 4, 6):
                for c in range(MT):
                    ops2[j * MT + c]()

    nc.compile()
    return nc


def _get_program():
    if "nc" not in _CACHE:
        _CACHE["nc"] = _build_program()
    return _CACHE["nc"]


def _per_core_inputs(x, y, mask, W_kv, b_kv, W_q, b_q, W_o):
    """Build the 8 per-core input maps."""
    in_maps = []
    mask_f = mask.astype(np.float32)
    for c in range(8):
        b, g = c // 2, c % 2
        gh = np.arange(g * 8, g * 8 + 8)
        k_cols = (gh[:, None] * 2 * HD + np.arange(HD)[None, :]).ravel()
        v_cols = k_cols + HD
        q_cols = slice(g * 512, (g + 1) * 512)
        in_maps.append({
            "xT": np.ascontiguousarray(x[b].T).astype(BF),
            "yT": np.ascontiguousarray(y[b].T).astype(BF),
            "maskT": np.ascontiguousarray(mask_f[b].T).astype(BF),
            "wk": np.ascontiguousarray(W_kv[:, k_cols]).astype(BF),
            "wv": np.ascontiguousarray(W_kv[:, v_cols]).astype(BF),
            "wq": np.ascontiguousarray(W_q[:, q_cols]).astype(BF),
            "wo": np.ascontiguousarray(W_o[q_cols, :]).astype(BF),
            "bk": b_kv[k_cols].astype(np.float32).reshape(512, 1),
            "bq": b_q[np.arange(g * 512, (g + 1) * 512)]
                  .astype(np.float32).reshape(512, 1),
        })
    return in_maps


def kernel(x, y, mask, W_kv, b_kv, W_q, b_q, W_o, b_o):
    from concourse import bass_utils

    x = np.asarray(x, np.float32)
    y = np.asarray(y, np.float32)
    mask = np.asarray(mask)
    W_kv = np.asarray(W_kv, np.float32)
    b_kv = np.asarray(b_kv, np.float32)
    W_q = np.asarray(W_q, np.float32)
    b_q = np.asarray(b_q, np.float32)
    W_o = np.asarray(W_o, np.float32)
    b_o = np.asarray(b_o, np.float32)

    nc = _get_program()
    in_maps = _per_core_inputs(x, y, mask, W_kv, b_kv, W_q, b_q, W_o)
    res = bass_utils.run_bass_kernel_spmd(nc, in_maps, core_ids=list(range(8)))

    # b_v folds into a constant row: attn rows sum to 1, so each head adds
    # b_v_h @ W_o_h to every output row; b_o adds on top.
    v_cols_all = (np.arange(H)[:, None] * 2 * HD + HD
                  + np.arange(HD)[None, :]).ravel()
    const_row = b_kv[v_cols_all].astype(np.float32) @ W_o + b_o

    out = np.empty((B, NQ, D), np.float32)
    for b in range(B):
        out[b] = (res.results[2 * b]["out"].astype(np.float32)
                  + res.results[2 * b + 1]["out"].astype(np.float32)
                  + const_row)
    return out


if __name__ == "__main__":
    import reference

    inputs = {k: np.asarray(v) for k, v in reference.setup_inputs().items()}
    got = kernel(**inputs)
    exp = np.asarray(reference.reference(**inputs))
    err = np.abs(got - exp)
    print("absmax rel err:", err.max() / np.abs(exp).max())
